# revision 3
# baseline (speedup 1.0000x reference)
"""Trainium2 Bass kernel for nn_NodeModel (GNN message passing), v3.

Math (see reference):
  mesh_agg = scatter_mean(mesh_edge_attr, mesh_dst, N)
  world_agg = scatter_mean(world_edge_attr, world_dst, N)
  h = relu(concat([x, mesh_agg, world_agg]) @ W1 + b1) @ W2 + b2
  out = x + LayerNorm(h) * gamma + beta

Strategy (v3 -- fully feature-major, 3 DMAs per batch):
  - Host: nodes globally sorted by (mesh_degree, snake(world_degree)) and
    packed into 784 windows of 128 nodes; windows are dealt to (core, slot)
    sorted by their max-degree profile so the 8 windows sharing a baked slot
    are nearly identical.  Edges land feature-major as per-slot planes
    [feat=partition, node lane], zero padded to the per-slot plane count
    baked into the single SPMD program, and pre-scaled by 1/deg(dst) so the
    device scatter-SUM directly yields the mean.  x is pre-transposed to
    bf16 feature-major host-side; the output is returned feature-major and
    un-permuted host-side.  The device never does indexed gathers or
    transposes.
  - Scatter-sum runs on the Tensor engine with the IDENTITY as stationary:
    each slot plane is one matmul accumulated into PSUM (fp32)
    feature-major -- the layout every later stage uses.  One contiguous
    accumulation group per PSUM tile.
  - MLP runs feature-major on the PE in bf16 (fp32 psum).
  - LayerNorm stays feature-major: per-node mean / mean-square come from two
    PE matmuls against a constant (1/128) matrix (result is the stat
    broadcast across all partitions); rsqrt on ACT; normalize via DVE
    tensor_tensor; bf16 residual add on GPSIMD; bf16 store.  In this layout
    gamma/beta/b1/b2 are per-partition columns (free via scalar ports).
  - Batches of 4 windows are software-pipelined
    (load(t) | scatter(t-1) | mlp+ln+store(t-2)); only 3 DMAs per batch
    (edges, xT, out) so the 8 DMAHW semaphore lanes recycle with slack.
  - All 8 cores run the same program on different data; host gathers and
    inverse-permutes the output.
"""

import os
import sys

import numpy as np

sys.path.insert(0, "/opt/trn_rl_repo")

import ml_dtypes

N_NODES = 100000
N_MESH = 600000
N_WORLD = 300000
D = 128
P = 128
C = 8  # cores
EPS = 1e-5
WPC = -(-N_NODES // (C * P))  # 98 windows per core
NW_TOT = C * WPC  # 784 global windows
NS = NW_TOT * P  # 100352 node slots
NB = 4  # windows per batch

BF16 = ml_dtypes.bfloat16

LAST_STATS = {}


# ----------------------------------------------------------------------------
# Host-side packing
# ----------------------------------------------------------------------------

def _batches(wpc=WPC):
    out = []
    b0 = 0
    while b0 < wpc:
        out.append((b0, min(NB, wpc - b0)))
        b0 += NB
    return out


def _pack(x, mesh_edge_attr, world_edge_attr, mesh_dst, world_dst):
    """Build per-core device buffers + metadata."""
    mesh_dst = np.asarray(mesh_dst).astype(np.int64)
    world_dst = np.asarray(world_dst).astype(np.int64)

    dm = np.bincount(mesh_dst, minlength=N_NODES)
    dw = np.bincount(world_dst, minlength=N_NODES)

    # node order: primary mesh degree, secondary world degree in snake
    # (alternating) direction so window-max world degree stays near-ideal at
    # mesh-degree class boundaries.
    sec = np.where(dm % 2 == 0, dw, dw.max() - dw)
    order = np.lexsort((sec, dm))
    pad = NS - N_NODES
    nw_tot = NW_TOT
    wpc = WPC
    ipos = np.empty(N_NODES, dtype=np.int64)
    ipos[order] = pad + np.arange(N_NODES)
    dms = np.zeros(NS, dtype=np.int64)
    dws = np.zeros(NS, dtype=np.int64)
    dms[pad:] = dm[order]
    dws[pad:] = dw[order]

    wmax_m = dms.reshape(nw_tot, P).max(axis=1)
    wmax_w = dws.reshape(nw_tot, P).max(axis=1)
    wsec = np.where(wmax_m % 2 == 0, wmax_w, wmax_w.max() - wmax_w)
    wrank = np.empty(nw_tot, dtype=np.int64)
    wrank[np.lexsort((wsec, wmax_m))] = np.arange(nw_tot)
    win_core = wrank % C          # [nw_tot]
    win_slot = wrank // C
    Tm = np.ones(wpc, np.int64)
    Tw = np.ones(wpc, np.int64)
    np.maximum.at(Tm, win_slot, np.maximum(wmax_m, 1))
    np.maximum.at(Tw, win_slot, np.maximum(wmax_w, 1))
    # per-slot plane layout: slot block = mesh planes then world planes
    coe = np.concatenate([[0], np.cumsum(P * (Tm + Tw))])  # len wpc+1
    com = coe[:-1]
    cow = coe[:-1] + P * Tm
    CDT = int(coe[-1])

    buf = np.zeros(C * P * CDT, dtype=BF16)

    rim = (1.0 / np.maximum(dm, 1)).astype(np.float32)
    riw = (1.0 / np.maximum(dw, 1)).astype(np.float32)

    def pack_edges(attr, dst, deg, rinv, co):
        M = dst.shape[0]
        perm = np.argsort(dst, kind="stable")
        starts = np.concatenate([[0], np.cumsum(deg)])
        dst_sorted = dst[perm]
        k = np.arange(M, dtype=np.int64) - starts[dst_sorted]
        i = ipos[dst_sorted]
        g = i // P
        n = i % P
        c = win_core[g]
        s = win_slot[g]
        base = c * (P * CDT) + co[s] + k * P + n
        attr = np.ascontiguousarray(attr, dtype=np.float32)
        scale = rinv[dst_sorted]
        d_ar = np.arange(D, dtype=np.int64) * CDT
        CH = 120000
        for lo in range(0, M, CH):
            hi = min(lo + CH, M)
            vals = (attr[perm[lo:hi]] * scale[lo:hi, None]).astype(BF16)
            idx = base[lo:hi, None] + d_ar[None, :]
            buf[idx] = vals

    pack_edges(mesh_edge_attr, mesh_dst, dm, rim, com)
    pack_edges(world_edge_attr, world_dst, dw, riw, cow)
    edge_buf = buf.reshape(C, P, CDT)

    # permuted xT per core: [C, D, wpc*P] bf16 feature-major
    i = ipos[order]
    g = i // P
    p = i % P
    c = win_core[g]
    s = win_slot[g]
    row = s * P + p

    x = np.ascontiguousarray(x, dtype=np.float32)
    x_perm = np.zeros((C, wpc * P, D), dtype=np.float32)
    x_perm[c, row] = x[order]
    xT_buf = np.ascontiguousarray(x_perm.transpose(0, 2, 1)).astype(BF16)

    unperm = (c, row)  # out[order] = result[c, row]
    return dict(
        Tm=Tm, Tw=Tw, coe=coe, CDT=CDT, edge_buf=edge_buf,
        xT_buf=xT_buf,
        order=order, unperm=unperm, wpc=wpc,
    )


# ----------------------------------------------------------------------------
# Device program
# ----------------------------------------------------------------------------

def _build_program(Tm, Tw, coe, CDT, has_beta, has_gamma=True, wpc=WPC):
    from contextlib import ExitStack
    import concourse.bass as bass
    import concourse.tile as tile
    from concourse import bacc, mybir

    f32 = mybir.dt.float32
    bf16 = mybir.dt.bfloat16
    AF = mybir.ActivationFunctionType
    OP = mybir.AluOpType

    nc = bacc.Bacc("TRN2", target_bir_lowering=False, debug=False,
                   enable_asserts=False, num_devices=C)

    edge_d = nc.dram_tensor("edge_buf", [P, CDT], bf16, kind="ExternalInput").ap()
    xT_d = nc.dram_tensor("xT_buf", [P, wpc * P], bf16, kind="ExternalInput").ap()
    w1a_d = nc.dram_tensor("w1a", [D, D], bf16, kind="ExternalInput").ap()
    w1b_d = nc.dram_tensor("w1b", [D, D], bf16, kind="ExternalInput").ap()
    w1c_d = nc.dram_tensor("w1c", [D, D], bf16, kind="ExternalInput").ap()
    w2_d = nc.dram_tensor("w2", [D, D], bf16, kind="ExternalInput").ap()
    # column-replicated W2 row-means: matmul(m2bc, h1s) broadcasts
    # mean_d(h2[d, n]) across all partitions
    m2_d = nc.dram_tensor("m2bc", [D, D], bf16, kind="ExternalInput").ap()
    b1_d = nc.dram_tensor("b1c", [P, 1], f32, kind="ExternalInput").ap()
    b2_d = nc.dram_tensor("b2c", [P, 1], f32, kind="ExternalInput").ap()
    if has_gamma or has_beta:
        gb_d = nc.dram_tensor("gamma_c", [P, 1], f32, kind="ExternalInput").ap()
        bb_d = nc.dram_tensor("beta_c", [P, 1], f32, kind="ExternalInput").ap()
    ident_d = nc.dram_tensor("ident", [P, P], bf16, kind="ExternalInput").ap()
    out_d = nc.dram_tensor("out_buf", [P, wpc * P], bf16,
                           kind="ExternalOutput").ap()

    bat = _batches(wpc)
    nbat = len(bat)

    with tile.TileContext(nc) as tc, ExitStack() as ctx:
        ctx.enter_context(nc.allow_low_precision(
            reason="bf16 intermediates are intentional; accumulation is fp32"))
        const = ctx.enter_context(tc.tile_pool(name="const", bufs=1))
        epool = ctx.enter_context(tc.tile_pool(name="edges", bufs=4))
        xTpool = ctx.enter_context(tc.tile_pool(name="xT", bufs=6))
        apool = ctx.enter_context(tc.tile_pool(name="aggs", bufs=4))
        hpool = ctx.enter_context(tc.tile_pool(name="hmid", bufs=3))
        spool = ctx.enter_context(tc.tile_pool(name="stats", bufs=2))
        opool = ctx.enter_context(tc.tile_pool(name="outw", bufs=3))
        psum = ctx.enter_context(tc.tile_pool(name="psumagg", bufs=4, space="PSUM"))
        psumh = ctx.enter_context(tc.tile_pool(name="psumh", bufs=2, space="PSUM"))
        psums = ctx.enter_context(tc.tile_pool(name="psums", bufs=1, space="PSUM"))

        def cload(shape, dt, src, tag):
            # ACT ring: keeps the SP ring free so edge(0) starts immediately
            t = const.tile(shape, dt, tag=tag)
            nc.scalar.dma_start(t[:], src)
            return t

        ident = cload([P, P], bf16, ident_d, "ident")
        w1a = cload([D, D], bf16, w1a_d, "w1a")
        w1b = cload([D, D], bf16, w1b_d, "w1b")
        w1c = cload([D, D], bf16, w1c_d, "w1c")
        w2 = cload([D, D], bf16, w2_d, "w2")
        m2bc = cload([D, D], bf16, m2_d, "m2bc")
        b1 = cload([P, 1], f32, b1_d, "b1")
        b2 = cload([P, 1], f32, b2_d, "b2")
        if has_gamma or has_beta:
            gc = cload([P, 1], f32, gb_d, "gc")
            bc = cload([P, 1], f32, bb_d, "bc")
        epsc = const.tile([P, 1], f32, tag="epsc")
        nc.gpsimd.memset(epsc[:], EPS)
        # (1/128) * ones: matmul against this broadcasts per-node means
        # across all 128 partitions.
        oneh = const.tile([P, P], bf16, tag="oneh")
        nc.gpsimd.memset(oneh[:], 1.0 / P)

        state = {}

        def stage_load(bi):
            """Issue edge/xT loads for batch bi."""
            s0, nb = bat[bi]
            col0, col1 = int(coe[s0]), int(coe[s0 + nb])

            eet = epool.tile([P, col1 - col0], bf16, tag="edges")
            nc.sync.dma_start(eet[:], edge_d[:, col0:col1])
            xTt = xTpool.tile([P, nb * D], bf16, tag="xT")
            nc.sync.dma_start(xTt[:], xT_d[:, s0 * P:(s0 + nb) * P])
            state[bi] = dict(eet=eet, xTt=xTt, nb=nb)

        def stage_scatter(bi):
            """Identity-stationary scatter into fp32 PSUM + agg copies.

            Accumulates feature-major: psum[d, j*128+n] += plane_k(slot j).
            One contiguous accumulation group per psum tile (hardware clears
            accumulation state per bank on start=True); disjoint column
            ranges accumulate independently via per-element has_written bits.
            """
            s0, nb = bat[bi]
            col0 = int(coe[s0])
            st = state[bi]
            eet = st.pop("eet")

            pm = psum.tile([P, nb * P], f32, tag="pagg")
            pw = psum.tile([P, nb * P], f32, tag="pagg")
            nm_tot = sum(int(Tm[s]) for s in range(s0, s0 + nb))
            nw_tot_ = sum(int(Tw[s]) for s in range(s0, s0 + nb))
            mi = 0
            for j in range(nb):
                s = s0 + j
                moff = int(coe[s]) - col0
                for si in range(int(Tm[s])):
                    nc.tensor.matmul(
                        pm[:, j * P:(j + 1) * P],
                        ident[:],
                        eet[:, moff + si * P:moff + (si + 1) * P],
                        start=(mi == 0), stop=(mi == nm_tot - 1),
                        skip_group_check=True,
                    )
                    mi += 1
            wi = 0
            for j in range(nb):
                s = s0 + j
                woff = int(coe[s]) - col0 + int(Tm[s]) * P
                for si in range(int(Tw[s])):
                    nc.tensor.matmul(
                        pw[:, j * P:(j + 1) * P],
                        ident[:],
                        eet[:, woff + si * P:woff + (si + 1) * P],
                        start=(wi == 0), stop=(wi == nw_tot_ - 1),
                        skip_group_check=True,
                    )
                    wi += 1
            # means are pre-scaled host-side; plain copies to bf16
            magg = apool.tile([P, nb * D], bf16, tag="magg")
            nc.scalar.activation(magg[:], pm[:], AF.Copy)
            wagg = apool.tile([P, nb * D], bf16, tag="wagg")
            nc.vector.tensor_scalar(wagg[:], pw[:], 1.0, None, op0=OP.mult)
            st["magg"] = magg
            st["wagg"] = wagg

        def stage_mlp(bi):
            """MLP feature-major, then center: t1 = y - mean(y), t1sq = t1^2.

            The per-node mean comes from a PE matmul against the constant
            (1/128)-matrix, which broadcasts it across all partitions.
            """
            s0, nb = bat[bi]
            st = state[bi]
            w = nb * D

            h1 = psumh.tile([P, w], f32, tag="h12")
            nc.tensor.matmul(h1[:], w1a[:], st["xTt"][:], start=True, stop=False)
            nc.tensor.matmul(h1[:], w1b[:], st.pop("magg")[:], start=False, stop=False)
            nc.tensor.matmul(h1[:], w1c[:], st.pop("wagg")[:], start=False, stop=True)
            h1s = hpool.tile([P, w], bf16, tag="h1s")
            nc.scalar.activation(h1s[:], h1[:], AF.Relu, bias=b1[:, 0:1])
            h2 = psumh.tile([P, w], f32, tag="h12")
            nc.tensor.matmul(h2[:], w2[:], h1s[:], start=True, stop=True)
            # per-node mean of h2, broadcast across partitions, straight from
            # h1s -- no dependency on the yT copy (b2 is mean-centered
            # host-side so its mean never enters)
            pmu = psums.tile([P, w], f32, tag="pmu")
            nc.tensor.matmul(pmu[:], m2bc[:], h1s[:], start=True, stop=True)
            yT = hpool.tile([P, w], bf16, tag="yT")
            nc.scalar.activation(yT[:], h2[:], AF.Identity, bias=b2[:, 0:1])
            t1 = hpool.tile([P, w], bf16, tag="t1")
            nc.vector.tensor_tensor(t1[:], yT[:], pmu[:], op=OP.subtract)
            t1sq = hpool.tile([P, w], bf16, tag="t1sq")
            nc.scalar.activation(t1sq[:], t1[:], AF.Square)
            st["t1"] = t1
            st["t1sq"] = t1sq

        def stage_tail(bi):
            """Variance via PE broadcast matmul + normalize + residual + store."""
            s0, nb = bat[bi]
            st = state.pop(bi)
            t1, t1sq, xTt = st["t1"], st["t1sq"], st["xTt"]
            w = nb * D

            pvar = psums.tile([P, w], f32, tag="pvar")
            nc.tensor.matmul(pvar[:], oneh[:], t1sq[:], start=True, stop=True)
            sd = spool.tile([P, w], f32, tag="sd")
            nc.scalar.activation(sd[:], pvar[:], AF.Sqrt, bias=epsc[:, 0:1])
            rstd = spool.tile([P, w], bf16, tag="rstd")
            nc.vector.reciprocal(rstd[:], sd[:])

            tn = hpool.tile([P, w], bf16, tag="tn")
            nc.vector.tensor_tensor(tn[:], t1[:], rstd[:], op=OP.mult)
            if has_gamma or has_beta:
                tg = hpool.tile([P, w], bf16, tag="tg")
                nc.vector.tensor_scalar(tg[:], tn[:], gc[:, 0:1], bc[:, 0:1],
                                        op0=OP.mult, op1=OP.add)
                tn = tg
            on = opool.tile([P, w], bf16, tag="on")
            nc.gpsimd.tensor_tensor(on[:], tn[:], xTt[:], op=OP.add)

            nc.scalar.dma_start(out_d[:, s0 * P:(s0 + nb) * P], on[:])

        # software-pipelined emission, oldest stage first within each step:
        #   LN+store(t-3) | MLP(t-2) | scatter(t-1) | load(t)
        # Each engine's FIFO then runs oldest batch first, and the PE stats
        # matmuls of batch b execute after scatter(b+2) with their ACT-made
        # inputs long ready -- no dependency bubbles on any engine.
        for t in range(nbat + 3):
            if t >= 3:
                stage_tail(t - 3)
            if 2 <= t < nbat + 2:
                stage_mlp(t - 2)
            if 1 <= t < nbat + 1:
                stage_scatter(t - 1)
            if t < nbat:
                stage_load(t)

    nc.compile()
    return nc


_PROGRAM_CACHE = {}


def _get_program(Tm, Tw, coe, CDT, has_beta, has_gamma, wpc=WPC):
    key = (tuple(Tm), tuple(Tw), bool(has_beta), bool(has_gamma), wpc)
    if key not in _PROGRAM_CACHE:
        _PROGRAM_CACHE[key] = _build_program(Tm, Tw, coe, CDT, has_beta,
                                             has_gamma, wpc)
    return _PROGRAM_CACHE[key]


# ----------------------------------------------------------------------------
# SPMD runner (PJRT over axon), with optional repeat timing
# ----------------------------------------------------------------------------

_RUNNER_CACHE = {}


def _make_runner(nc):
    import jax
    from jax.sharding import Mesh, PartitionSpec, NamedSharding
    from jax.experimental.shard_map import shard_map
    from concourse import mybir
    from concourse.bass2jax import (_bass_exec_p, install_neuronx_cc_hook,
                                    partition_id_tensor)

    install_neuronx_cc_hook()

    partition_name = (nc.partition_id_tensor.name
                      if nc.partition_id_tensor else None)
    in_names, out_names, out_avals = [], [], []
    for alloc in nc.m.functions[0].allocations:
        if not isinstance(alloc, mybir.MemoryLocationSet):
            continue
        name = alloc.memorylocations[0].name
        if alloc.kind == "ExternalInput":
            if name != partition_name:
                in_names.append(name)
        elif alloc.kind == "ExternalOutput":
            out_names.append(name)
            out_avals.append(jax.core.ShapedArray(
                tuple(alloc.tensor_shape), mybir.dt.np(alloc.dtype)))
    n_params = len(in_names)
    all_names = in_names + out_names
    if partition_name is not None:
        all_names = all_names + [partition_name]

    def _body(*args):
        operands = list(args)
        if partition_name is not None:
            operands.append(partition_id_tensor())
        outs = _bass_exec_p.bind(
            *operands,
            out_avals=tuple(out_avals),
            in_names=tuple(all_names),
            out_names=tuple(out_names),
            lowering_input_output_aliases=(),
            sim_require_finite=True,
            sim_require_nnan=True,
            nc=nc,
        )
        return tuple(outs)

    devices = jax.devices()[:C]
    mesh = Mesh(np.asarray(devices), ("core",))
    spec = PartitionSpec("core")
    n_out = len(out_names)
    fn = jax.jit(
        shard_map(_body, mesh=mesh,
                  in_specs=(spec,) * (n_params + n_out),
                  out_specs=(spec,) * n_out,
                  check_rep=False),
        keep_unused=True,
    )
    sharding = NamedSharding(mesh, spec)
    return fn, in_names, out_names, out_avals, sharding


def _run_spmd(nc, in_maps, time_iters=0):
    import jax
    import time

    key = id(nc)
    if key not in _RUNNER_CACHE:
        _RUNNER_CACHE[key] = _make_runner(nc)
    fn, in_names, out_names, out_avals, sharding = _RUNNER_CACHE[key]

    concat_in = [
        jax.device_put(
            np.concatenate([np.asarray(in_maps[c][n]) for c in range(C)], axis=0),
            sharding)
        for n in in_names
    ]
    concat_zero = [
        jax.device_put(np.zeros((C * a.shape[0], *a.shape[1:]), a.dtype), sharding)
        for a in out_avals
    ]
    args = concat_in + concat_zero
    out = fn(*args)
    jax.block_until_ready(out)

    if time_iters > 0:
        for _ in range(10):  # reach steady-state dispatch before timing
            out = fn(*args)
        jax.block_until_ready(out)
        t0 = time.perf_counter()
        for _ in range(time_iters):
            out = fn(*args)
        jax.block_until_ready(out)
        t1 = time.perf_counter()
        LAST_STATS["wall_per_iter_ns"] = (t1 - t0) / time_iters * 1e9
        times = []
        for _ in range(time_iters):
            t0 = time.perf_counter()
            jax.block_until_ready(fn(*args))
            times.append(time.perf_counter() - t0)
        LAST_STATS["wall_min_ns"] = min(times) * 1e9

    return [
        {n: np.asarray(out[i]).reshape(C, *out_avals[i].shape)[c]
         for i, n in enumerate(out_names)}
        for c in range(C)
    ]


# ----------------------------------------------------------------------------
# Entry point
# ----------------------------------------------------------------------------

def kernel(x, mesh_edge_attr, world_edge_attr, mesh_dst, world_dst,
           W1, b1, W2, b2, gamma, beta):
    x = np.asarray(x, dtype=np.float32)
    W1 = np.asarray(W1, dtype=np.float32)
    W2 = np.asarray(W2, dtype=np.float32)
    b1 = np.asarray(b1, dtype=np.float32)
    b2 = np.asarray(b2, dtype=np.float32)
    gamma = np.asarray(gamma, dtype=np.float32)
    beta = np.asarray(beta, dtype=np.float32)

    pk = _pack(x, np.asarray(mesh_edge_attr, dtype=np.float32),
               np.asarray(world_edge_attr, dtype=np.float32),
               mesh_dst, world_dst)

    has_beta = bool(np.any(beta != 0.0))
    has_gamma = not bool(np.all(gamma == 1.0))
    nc = _get_program(pk["Tm"], pk["Tw"], pk["coe"], pk["CDT"], has_beta,
                      has_gamma, wpc=pk["wpc"])

    w1a = np.ascontiguousarray(W1[0:D]).astype(BF16)
    w1b = np.ascontiguousarray(W1[D:2 * D]).astype(BF16)
    w1c = np.ascontiguousarray(W1[2 * D:3 * D]).astype(BF16)
    w2 = np.ascontiguousarray(W2).astype(BF16)
    # mean over output features of h2 = W2.T @ h1s: m2[k] = mean_d W2[k, d];
    # replicate as columns so matmul(m2bc, h1s) broadcasts the mean.
    m2 = W2.mean(axis=1).astype(np.float32)
    m2bc = np.ascontiguousarray(np.tile(m2[:, None], (1, D))).astype(BF16)
    b1c = np.ascontiguousarray(b1.reshape(P, 1))
    # center b2: LayerNorm subtracts the per-node mean, so only
    # (b2 - mean(b2)) ever survives -- bake the centering into the bias.
    b2c = np.ascontiguousarray((b2 - b2.mean()).reshape(P, 1))
    ident = np.eye(P, dtype=BF16)

    in_maps = []
    for c in range(C):
        m = {
            "edge_buf": pk["edge_buf"][c],
            "xT_buf": pk["xT_buf"][c],
            "w1a": w1a, "w1b": w1b, "w1c": w1c, "w2": w2, "m2bc": m2bc,
            "b1c": b1c, "b2c": b2c, "ident": ident,
        }
        if has_gamma or has_beta:
            m["gamma_c"] = np.ascontiguousarray(gamma.reshape(P, 1))
            m["beta_c"] = np.ascontiguousarray(beta.reshape(P, 1))
        in_maps.append(m)

    results = _run_spmd(nc, in_maps,
                        time_iters=int(os.environ.get("KERNEL_TIME_ITERS", "0")))

    out_stack = np.stack([results[c]["out_buf"] for c in range(C)])  # [C,P,wpc*P]
    out_nodes = out_stack.transpose(0, 2, 1).astype(np.float32)       # [C,wpc*P,P]
    c_idx, row_idx = pk["unperm"]
    out = np.empty((N_NODES, D), dtype=np.float32)
    out[pk["order"]] = out_nodes[c_idx, row_idx]
    return out
